# revision 27
# baseline (speedup 1.0000x reference)
"""GQA (B=2, S=2048, d_model=2048, 16 Q heads / 4 KV groups) + output projection.

Sharding: 8 cores, core c <-> (b = c//4, g = c%4). Each core computes full
attention for the 4 query heads of KV group g of batch b, then multiplies its
512-feature slice of the concatenated head outputs with the matching 512 rows
of Wc^T, producing a partial [S, d_model] projection. Host sums the 4 partials
per batch element (bias is folded into the g==0 core's partial).

On-core layout: everything transposed, all matmul operands bf16 (full PE rate,
LDWEIGHTS at fast-weight-load rate so it hides behind the 512-col matmuls;
fp32 LDWEIGHTS costs ~213ns = a full matmul and cannot hide).
  scoresT[t, s] = kT.T @ qT           (lhsT = kT tile [d,128t], rhs = qT [d,512s])
  expT = exp(scoresT / sqrt(128))     (ACT, fused scale, f32 PSUM in / bf16 out,
                                       no max subtraction: scores ~ N(0,1))
  tree: 3 pairwise levels on DVE      (bf16 2x mode; 14 adds reduce the 16
                                       t-tiles to 2, so the softmax-sum matmul
                                       below costs 2x512 rows instead of 16x512)
  sums[1, s]  = ones.T @ tree out     (PE, 2 accumulating matmuls)
  uT[hd, s]   = v.T @ expT            (PE, accumulated over 16 t tiles)
  attnT = uT * bcast(1 / sums)        (DVE recip + GPSIMD partition_broadcast
                                       + DVE mult, attnT stored bf16)
  out[s, o]   = attnT.T @ wT + bias   (PE, contraction over the 512 features,
                                       bias folded into the DVE PSUM->SBUF move)

DMA: everything on the sync hardware-DGE ring (the gpsimd software DGE takes
~10us to emit its first packet and drip-feeds strided transfers). The PE's
first matmul waits on the ring's shared completion counter, i.e. on ALL DMAs
issued before it -- so only the two tiles the first QK chain reads (kT chunk 0,
qT(0)) are issued ahead of it; v/wT/bias and the next qT tiles are issued right
after QK(0) is emitted. v and wT are host-prepacked into their exact SBUF
layouts so each is one contiguous descriptor-cheap transfer.

Scheduling: software-pipelined combos k = (s_block j, head h). Slot k emits
QK(k) then tree(k-1) / PV(k-1) / sums(k-1) / normalize(k-1); proj(j) is emitted
one full combo after group j finishes so the normalize chain never stalls the
PE (PE gaps > ~3.4us re-throttle the HAM clock gate to half speed). Projection
output DMAs go out per [128,512] chunk so the final transfer after the last
matmul is small.
"""

import math
import sys

sys.path.insert(0, "/opt/trn_rl_repo")

import ml_dtypes
import numpy as np

import concourse.bacc as bacc
import concourse.bass as bass
import concourse.mybir as mybir
import concourse.tile as tile
from concourse.bass import ds, ts
from concourse.bass_utils import run_bass_kernel_spmd

F32 = mybir.dt.float32
BF16 = mybir.dt.bfloat16

B = 2
S = 2048
D_MODEL = 2048
N_GROUPS = 4
HEADS_PER_GROUP = 4
HEAD_DIM = 128
P = 128
NT = S // P          # 16 t tiles
NJ = S // 512        # 4 s blocks
SCALE = 1.0 / math.sqrt(HEAD_DIM)

_COMPILED = None


def _build():
    nc = bacc.Bacc(None, target_bir_lowering=False)

    qT_d = nc.dram_tensor("qT", [P, HEADS_PER_GROUP, S], BF16, kind="ExternalInput")
    kT_d = nc.dram_tensor("kT", [P, S], BF16, kind="ExternalInput")
    v_d = nc.dram_tensor("v", [P, NT, P], BF16, kind="ExternalInput")
    wT_d = nc.dram_tensor("wT", [P, HEADS_PER_GROUP, D_MODEL], BF16, kind="ExternalInput")
    out_d = nc.dram_tensor("out", [S, D_MODEL], F32, kind="ExternalOutput")

    Exp = mybir.ActivationFunctionType.Exp
    mult = mybir.AluOpType.mult
    add = mybir.AluOpType.add

    with tile.TileContext(nc) as tc:
        with (
            tc.tile_pool(name="const", bufs=1) as const_pool,
            tc.tile_pool(name="qt", bufs=3) as qt_pool,
            tc.tile_pool(name="expT", bufs=3) as expT_pool,
            tc.tile_pool(name="tree", bufs=2) as tree_pool,
            tc.tile_pool(name="attnT", bufs=8) as attnT_pool,
            tc.tile_pool(name="small", bufs=2) as small_pool,
            tc.tile_pool(name="orow", bufs=4) as orow_pool,
            tc.tile_pool(name="qk_ps", bufs=2, space="PSUM") as qk_psum,
            tc.tile_pool(name="acc_ps", bufs=4, space="PSUM") as acc_psum,
        ):
            # All-ones [128,128] stationary: the softmax-sum matmul then
            # writes the sum to every output partition (same cost -- matmul
            # cost is moving rows only), so no partition_broadcast is needed.
            ones_mat = const_pool.tile([P, P], BF16, tag="ones_mat")
            nc.vector.memset(ones_mat[:], 1.0)

            # Only the first QK combo's data ahead of the first matmul: the
            # PE waits on the sync ring's shared DMA-completion counter, so
            # anything issued before QK(0) delays its first matmul.
            kT_chunks = []
            for c in range(4):
                kc = const_pool.tile([P, 512], BF16, tag=f"kT{c}")
                kT_chunks.append(kc)
            nc.sync.dma_start(kT_chunks[0][:], kT_d[:, ts(0, 512)])
            qt0 = qt_pool.tile([P, 512], BF16, tag="qT")
            nc.sync.dma_start(qt0[:], qT_d[:, 0, ts(0, 512)])
            for c in range(1, 4):
                nc.sync.dma_start(kT_chunks[c][:], kT_d[:, ts(c, 512)])

            v_sb = const_pool.tile([P, NT, P], BF16, tag="v")
            wT_sb = const_pool.tile([P, HEADS_PER_GROUP, D_MODEL], BF16, tag="wT")

            qt_early = {0: qt0}
            expT_tiles = {}
            tree_tiles = {}
            attnT_tiles = {}

            def emit_qk(k):
                j, h = divmod(k, HEADS_PER_GROUP)
                if k in qt_early:
                    qt = qt_early[k]
                else:
                    qt = qt_pool.tile([P, 512], BF16, tag="qT")
                    nc.sync.dma_start(qt[:], qT_d[:, h, ts(j, 512)])
                et_all = expT_pool.tile([P, NT, 512], BF16, tag="expT")
                for pp in range(NT // 2):
                    ps = qk_psum.tile([P, 2, 512], F32, tag="qk")
                    for u in range(2):
                        tt = pp * 2 + u
                        nc.tensor.matmul(
                            ps[:, u, :], kT_chunks[tt // 4][:, ts(tt % 4, P)], qt[:],
                            start=True, stop=True,
                        )
                    nc.scalar.activation(
                        et_all[:, ds(pp * 2, 2), :], ps[:], Exp, scale=SCALE
                    )
                expT_tiles[k] = et_all

            def emit_bulk_loads():
                # Issued after QK(0)'s matmuls so they don't gate the first MM;
                # ordered by first use: v (PV(0)), then the next q tiles, then
                # wT (first used by proj(0) ~50us in).
                nc.sync.dma_start(v_sb[:], v_d[:])
                qt1 = qt_pool.tile([P, 512], BF16, tag="qT")
                nc.sync.dma_start(qt1[:], qT_d[:, 1, ts(0, 512)])
                qt2 = qt_pool.tile([P, 512], BF16, tag="qT")
                nc.sync.dma_start(qt2[:], qT_d[:, 2, ts(0, 512)])
                nc.sync.dma_start(wT_sb[:], wT_d[:])
                qt_early[1] = qt1
                qt_early[2] = qt2

            def emit_tree(k):
                # Four pairwise levels on DVE (bf16 2x): 16 -> 8 -> 4 -> 2 -> 1.
                et_all = expT_tiles[k]
                t1 = tree_pool.tile([P, 8, 512], BF16, tag="tree1")
                for i in range(8):
                    nc.vector.tensor_tensor(
                        t1[:, i, :], et_all[:, i, :], et_all[:, i + 8, :], add
                    )
                t2 = tree_pool.tile([P, 4, 512], BF16, tag="tree2")
                for i in range(4):
                    nc.vector.tensor_tensor(
                        t2[:, i, :], t1[:, i, :], t1[:, i + 4, :], add
                    )
                t3 = tree_pool.tile([P, 2, 512], BF16, tag="tree3")
                for i in range(2):
                    nc.vector.tensor_tensor(
                        t3[:, i, :], t2[:, i, :], t2[:, i + 2, :], add
                    )
                t4 = tree_pool.tile([P, 512], BF16, tag="tree4")
                nc.vector.tensor_tensor(t4[:], t3[:, 0, :], t3[:, 1, :], add)
                tree_tiles[k] = t4

            def emit_pv(k):
                et_all = expT_tiles[k]
                pv_ps = acc_psum.tile([P, 512], F32, tag="acc")
                for tt in range(NT):
                    nc.tensor.matmul(
                        pv_ps[:], v_sb[:, tt, :], et_all[:, tt, :],
                        start=(tt == 0), stop=(tt == NT - 1),
                    )
                return pv_ps

            def emit_sums(k):
                sum_ps = acc_psum.tile([P, 512], F32, tag="acc")
                if k in tree_tiles:
                    t4 = tree_tiles[k]
                    nc.tensor.matmul(
                        sum_ps[:], ones_mat[:], t4[:], start=True, stop=True
                    )
                else:
                    # Last combo: sum the 16 expT tiles directly on the PE
                    # (dense matmuls, HAM stays warm) instead of waiting on
                    # the DVE tree -- it would gate the whole tail chain.
                    et_all = expT_tiles[k]
                    for tt in range(NT):
                        nc.tensor.matmul(
                            sum_ps[:], ones_mat[:], et_all[:, tt, :],
                            start=(tt == 0), stop=(tt == NT - 1),
                        )
                return sum_ps

            def emit_norm(k, sum_ps, pv_ps):
                j, h = divmod(k, HEADS_PER_GROUP)
                rb_bc = small_pool.tile([P, 512], F32, tag="rb_bc")
                nc.vector.reciprocal_approx_fast(rb_bc[:], sum_ps[:])
                at = attnT_pool.tile([P, 512], BF16, tag="attnT")
                nc.vector.tensor_tensor(at[:], pv_ps[:], rb_bc[:], mult)
                attnT_tiles[(j, h)] = at
                del expT_tiles[k]
                tree_tiles.pop(k, None)

            def emit_proj(j, st_list, drain_engine="vector"):
                # bias is added on the host; the PSUM drain is a plain copy,
                # which the (tail-idle) ACT engine can take for the last group
                # so the final drains overlap the final matmuls.
                for st in st_list:
                    for ob in range(4):
                        po = acc_psum.tile([P, 512], F32, tag="acc")
                        for h in range(HEADS_PER_GROUP):
                            nc.tensor.matmul(
                                po[:], attnT_tiles[(j, h)][:, ts(st, P)],
                                wT_sb[:, h, ts(ob, 512)],
                                start=(h == 0), stop=(h == HEADS_PER_GROUP - 1),
                            )
                        orow = orow_pool.tile([P, 512], F32, tag="orow")
                        if drain_engine == "scalar":
                            nc.scalar.copy(orow[:], po[:])
                        else:
                            nc.vector.tensor_copy(orow[:], po[:])
                        nc.sync.dma_start(
                            out_d[ds(j * 512 + st * P, P), ts(ob, 512)], orow[:]
                        )

            n_combos = NJ * HEADS_PER_GROUP
            for k in range(n_combos + 2):
                if k < n_combos:
                    emit_qk(k)
                if k == 0:
                    emit_bulk_loads()
                if 1 <= k <= n_combos:
                    if k - 1 < n_combos - 1:
                        emit_tree(k - 1)
                    pv_ps = emit_pv(k - 1)
                    sum_ps = emit_sums(k - 1)
                    emit_norm(k - 1, sum_ps, pv_ps)
                # proj(j) spread as one st-quarter per slot (slots j*4+4 ..
                # j*4+7) so every mid-run slot carries the same PE load and
                # the PE never outpaces the exp stream (a >3.4us PE gap
                # re-throttles the HAM clock gate). The last group runs
                # monolithically in slot 16: dense back-to-back matmuls keep
                # the clock warm through the tail.
                if 4 <= k < 16:
                    emit_proj((k - 4) // 4, [(k - 4) % 4])
                if k == 16:
                    emit_proj(3, [0, 1, 2, 3], drain_engine="scalar")

    nc.compile()
    return nc


def _get_nc():
    global _COMPILED
    if _COMPILED is None:
        _COMPILED = _build()
    return _COMPILED


def _shard_inputs(q, k, v, Wc, bc):
    in_maps = []
    for c in range(8):
        b, g = divmod(c, 4)
        qT = np.ascontiguousarray(
            q[b][:, g * 512:(g + 1) * 512].reshape(S, HEADS_PER_GROUP, P).transpose(2, 1, 0)
        ).astype(ml_dtypes.bfloat16)
        kT = np.ascontiguousarray(k[b][:, g * P:(g + 1) * P].T).astype(ml_dtypes.bfloat16)
        # v prepacked to the SBUF layout [p, tile, hd]: (p, n, d) = v[n*128+p, d]
        vv = np.ascontiguousarray(
            v[b][:, g * P:(g + 1) * P].reshape(NT, P, P).transpose(1, 0, 2)
        ).astype(ml_dtypes.bfloat16)
        # wT prepacked to [p, chunk, out]: (p, n, o) = Wc[o, g*512 + n*128 + p]
        wT = np.ascontiguousarray(
            Wc[:, g * 512:(g + 1) * 512].T.reshape(HEADS_PER_GROUP, P, D_MODEL).transpose(1, 0, 2)
        ).astype(ml_dtypes.bfloat16)
        in_maps.append({"qT": qT, "kT": kT, "v": vv, "wT": wT})
    return in_maps


def _run(inputs, trace=False):
    q = np.asarray(inputs["q"], dtype=np.float32)
    k = np.asarray(inputs["k"], dtype=np.float32)
    v = np.asarray(inputs["v"], dtype=np.float32)
    Wc = np.asarray(inputs["Wc"], dtype=np.float32)
    bc = np.asarray(inputs["bc"], dtype=np.float32)

    nc = _get_nc()
    in_maps = _shard_inputs(q, k, v, Wc, bc)
    res = run_bass_kernel_spmd(nc, in_maps, list(range(8)), trace=trace)

    out = np.empty((B, S, D_MODEL), dtype=np.float32)
    for b in range(B):
        acc = res.results[4 * b]["out"].astype(np.float32).copy()
        for g in range(1, 4):
            acc += res.results[4 * b + g]["out"]
        out[b] = acc + bc[None, :]
    return out, res


def kernel(**inputs):
    out, _ = _run(inputs, trace=False)
    return out


# revision 29
# speedup vs baseline: 1.0094x; 1.0094x over previous
"""GQA (B=2, S=2048, d_model=2048, 16 Q heads / 4 KV groups) + output projection.

Sharding: 8 cores, core c <-> (b = c//4, g = c%4). Each core computes full
attention for the 4 query heads of KV group g of batch b, then multiplies its
512-feature slice of the concatenated head outputs with the matching 512 rows
of Wc^T, producing a partial [S, d_model] projection. Host sums the 4 partials
per batch element and adds the bias.

On-core layout: everything transposed, all matmul operands bf16 (full PE rate,
LDWEIGHTS at fast-weight-load rate so it hides behind the 512-col matmuls;
fp32 LDWEIGHTS costs ~213ns = a full matmul and cannot hide).
  scoresT[t, s] = kT.T @ qT           (lhsT = kT tile [d,128t], rhs = qT [d,512s])
  expT = exp(scoresT / sqrt(128))     (ACT, fused scale, f32 PSUM in / bf16 out,
                                       no max subtraction: scores ~ N(0,1))
  tree: 4 pairwise levels on DVE      (bf16 2x mode; 15 adds reduce the 16
                                       t-tiles to 1, so the softmax-sum matmul
                                       below costs 512 rows instead of 16x512)
  sums[:, s] = ones128.T @ tree out   (PE; the all-ones [128,128] stationary
                                       broadcasts the sum to every output
                                       partition for free -- matmul cost is
                                       moving rows only -- so no gpsimd
                                       partition_broadcast is needed)
  uT[hd, s]   = v.T @ expT            (PE, accumulated over 16 t tiles)
  attnT = uT * (1 / sums)             (DVE recip + DVE mult, attnT stored bf16)
  out[s, o]   = attnT.T @ wT          (PE, contraction over the 512 features;
                                       drain PSUM->SBUF is a plain copy)

DMA: everything on the sync hardware-DGE ring (the gpsimd software DGE takes
~10us to emit its first packet and drip-feeds strided transfers). The PE's
first matmul waits on the ring's shared completion counter, i.e. on ALL DMAs
issued before it -- so only what the first QK combo reads (kT, qT(0)) is
issued ahead of it; v/wT and the next qT tiles are issued right after QK(0)
is emitted. v and wT are host-prepacked into their exact SBUF layouts so each
is one contiguous descriptor-cheap transfer.

Scheduling: software-pipelined combos k = (s_block j, head h). Slot k emits
QK(k) then tree(k-1) / PV(k-1) / sums(k-1) / normalize(k-1), plus one
st-quarter of proj(j) for the group finished 1..4 slots ago -- a uniform 49
matmuls per mid-run slot, so the PE never idles long enough (>3.4us) for the
HAM clock gate to re-throttle it to half speed. The last group instead runs
monolithically right after the last combo, its softmax sum is taken directly
over the 16 expT tiles on the PE (dense matmuls instead of the DVE tree's
latency), and its PSUM drains go to the by-then-idle ACT engine so the final
output chunks leave while the last matmuls still stream.
"""

import math
import sys

sys.path.insert(0, "/opt/trn_rl_repo")

import ml_dtypes
import numpy as np

import concourse.bacc as bacc
import concourse.bass as bass
import concourse.mybir as mybir
import concourse.tile as tile
from concourse.bass import ds, ts
from concourse.bass_utils import run_bass_kernel_spmd

F32 = mybir.dt.float32
BF16 = mybir.dt.bfloat16

B = 2
S = 2048
D_MODEL = 2048
N_GROUPS = 4
HEADS_PER_GROUP = 4
HEAD_DIM = 128
P = 128
NT = S // P          # 16 t tiles
NJ = S // 512        # 4 s blocks
SCALE = 1.0 / math.sqrt(HEAD_DIM)

_COMPILED = None


def _build():
    nc = bacc.Bacc(None, target_bir_lowering=False)

    qT_d = nc.dram_tensor("qT", [P, HEADS_PER_GROUP, S], BF16, kind="ExternalInput")
    kT_d = nc.dram_tensor("kT", [P, S], BF16, kind="ExternalInput")
    v_d = nc.dram_tensor("v", [P, NT, P], BF16, kind="ExternalInput")
    wT_d = nc.dram_tensor("wT", [P, HEADS_PER_GROUP, D_MODEL], BF16, kind="ExternalInput")
    out_d = nc.dram_tensor("out", [S, D_MODEL], F32, kind="ExternalOutput")

    Exp = mybir.ActivationFunctionType.Exp
    mult = mybir.AluOpType.mult
    add = mybir.AluOpType.add

    with tile.TileContext(nc) as tc:
        with (
            tc.tile_pool(name="const", bufs=1) as const_pool,
            tc.tile_pool(name="qt", bufs=3) as qt_pool,
            tc.tile_pool(name="expT", bufs=3) as expT_pool,
            tc.tile_pool(name="tree", bufs=2) as tree_pool,
            tc.tile_pool(name="attnT", bufs=8) as attnT_pool,
            tc.tile_pool(name="small", bufs=2) as small_pool,
            tc.tile_pool(name="orow", bufs=8) as orow_pool,
            tc.tile_pool(name="qk_ps", bufs=2, space="PSUM") as qk_psum,
            tc.tile_pool(name="acc_ps", bufs=4, space="PSUM") as acc_psum,
        ):
            # All-ones [128,128] stationary: the softmax-sum matmul then
            # writes the sum to every output partition (same cost -- matmul
            # cost is moving rows only), so no partition_broadcast is needed.
            ones_mat = const_pool.tile([P, P], BF16, tag="ones_mat")
            nc.vector.memset(ones_mat[:], 1.0)

            # Only the first QK combo's data ahead of the first matmul: the
            # PE waits on the sync ring's shared DMA-completion counter, so
            # anything issued before QK(0) delays its first matmul.
            kT_chunks = []
            for c in range(4):
                kc = const_pool.tile([P, 512], BF16, tag=f"kT{c}")
                kT_chunks.append(kc)
            nc.sync.dma_start(kT_chunks[0][:], kT_d[:, ts(0, 512)])
            qt0 = qt_pool.tile([P, 512], BF16, tag="qT")
            nc.sync.dma_start(qt0[:], qT_d[:, 0, ts(0, 512)])
            for c in range(1, 4):
                nc.sync.dma_start(kT_chunks[c][:], kT_d[:, ts(c, 512)])

            v_sb = const_pool.tile([P, NT, P], BF16, tag="v")
            wT_sb = const_pool.tile([P, HEADS_PER_GROUP, D_MODEL], BF16, tag="wT")

            qt_early = {0: qt0}
            expT_tiles = {}
            tree_tiles = {}
            attnT_tiles = {}

            def emit_qk(k):
                j, h = divmod(k, HEADS_PER_GROUP)
                if k in qt_early:
                    qt = qt_early[k]
                else:
                    qt = qt_pool.tile([P, 512], BF16, tag="qT")
                    nc.sync.dma_start(qt[:], qT_d[:, h, ts(j, 512)])
                et_all = expT_pool.tile([P, NT, 512], BF16, tag="expT")
                for pp in range(NT // 2):
                    ps = qk_psum.tile([P, 2, 512], F32, tag="qk")
                    for u in range(2):
                        tt = pp * 2 + u
                        nc.tensor.matmul(
                            ps[:, u, :], kT_chunks[tt // 4][:, ts(tt % 4, P)], qt[:],
                            start=True, stop=True,
                        )
                    nc.scalar.activation(
                        et_all[:, ds(pp * 2, 2), :], ps[:], Exp, scale=SCALE
                    )
                expT_tiles[k] = et_all

            def emit_bulk_loads():
                # Issued after QK(0)'s matmuls so they don't gate the first MM;
                # ordered by first use: v (PV(0)), then the next q tiles, then
                # wT (first used by proj(0) ~50us in).
                nc.sync.dma_start(v_sb[:], v_d[:])
                qt1 = qt_pool.tile([P, 512], BF16, tag="qT")
                nc.sync.dma_start(qt1[:], qT_d[:, 1, ts(0, 512)])
                qt2 = qt_pool.tile([P, 512], BF16, tag="qT")
                nc.sync.dma_start(qt2[:], qT_d[:, 2, ts(0, 512)])
                nc.sync.dma_start(wT_sb[:], wT_d[:])
                qt_early[1] = qt1
                qt_early[2] = qt2

            def emit_tree(k):
                # Four pairwise levels on DVE (bf16 2x): 16 -> 8 -> 4 -> 2 -> 1.
                et_all = expT_tiles[k]
                t1 = tree_pool.tile([P, 8, 512], BF16, tag="tree1")
                for i in range(8):
                    nc.vector.tensor_tensor(
                        t1[:, i, :], et_all[:, i, :], et_all[:, i + 8, :], add
                    )
                t2 = tree_pool.tile([P, 4, 512], BF16, tag="tree2")
                for i in range(4):
                    nc.vector.tensor_tensor(
                        t2[:, i, :], t1[:, i, :], t1[:, i + 4, :], add
                    )
                t3 = tree_pool.tile([P, 2, 512], BF16, tag="tree3")
                for i in range(2):
                    nc.vector.tensor_tensor(
                        t3[:, i, :], t2[:, i, :], t2[:, i + 2, :], add
                    )
                t4 = tree_pool.tile([P, 512], BF16, tag="tree4")
                nc.vector.tensor_tensor(t4[:], t3[:, 0, :], t3[:, 1, :], add)
                tree_tiles[k] = t4

            def emit_pv(k):
                et_all = expT_tiles[k]
                pv_ps = acc_psum.tile([P, 512], F32, tag="acc")
                for tt in range(NT):
                    nc.tensor.matmul(
                        pv_ps[:], v_sb[:, tt, :], et_all[:, tt, :],
                        start=(tt == 0), stop=(tt == NT - 1),
                    )
                return pv_ps

            def emit_sums(k):
                sum_ps = acc_psum.tile([P, 512], F32, tag="acc")
                if k in tree_tiles:
                    t4 = tree_tiles[k]
                    nc.tensor.matmul(
                        sum_ps[:], ones_mat[:], t4[:], start=True, stop=True
                    )
                else:
                    # Last combo: sum the 16 expT tiles directly on the PE
                    # (dense matmuls, HAM stays warm) instead of waiting on
                    # the DVE tree -- it would gate the whole tail chain.
                    et_all = expT_tiles[k]
                    for tt in range(NT):
                        nc.tensor.matmul(
                            sum_ps[:], ones_mat[:], et_all[:, tt, :],
                            start=(tt == 0), stop=(tt == NT - 1),
                        )
                return sum_ps

            def emit_norm(k, sum_ps, pv_ps):
                j, h = divmod(k, HEADS_PER_GROUP)
                rb_bc = small_pool.tile([P, 512], F32, tag="rb_bc")
                nc.vector.reciprocal_approx_fast(rb_bc[:], sum_ps[:])
                at = attnT_pool.tile([P, 512], BF16, tag="attnT")
                nc.vector.tensor_tensor(at[:], pv_ps[:], rb_bc[:], mult)
                attnT_tiles[(j, h)] = at
                del expT_tiles[k]
                tree_tiles.pop(k, None)

            def emit_proj(j, st_list, drain_engine="vector"):
                # bias is added on the host; the PSUM drain is a plain copy,
                # which the (tail-idle) ACT engine can take for the last group
                # so the final drains overlap the final matmuls.
                for st in st_list:
                    for ob in range(4):
                        po = acc_psum.tile([P, 512], F32, tag="acc")
                        for h in range(HEADS_PER_GROUP):
                            nc.tensor.matmul(
                                po[:], attnT_tiles[(j, h)][:, ts(st, P)],
                                wT_sb[:, h, ts(ob, 512)],
                                start=(h == 0), stop=(h == HEADS_PER_GROUP - 1),
                            )
                        orow = orow_pool.tile([P, 512], F32, tag="orow")
                        if drain_engine == "scalar":
                            nc.scalar.copy(orow[:], po[:])
                        else:
                            nc.vector.tensor_copy(orow[:], po[:])
                        nc.sync.dma_start(
                            out_d[ds(j * 512 + st * P, P), ts(ob, 512)], orow[:]
                        )

            n_combos = NJ * HEADS_PER_GROUP
            for k in range(n_combos + 2):
                if k < n_combos:
                    emit_qk(k)
                if k == 0:
                    emit_bulk_loads()
                if 1 <= k <= n_combos:
                    if k - 1 < n_combos - 1:
                        emit_tree(k - 1)
                    pv_ps = emit_pv(k - 1)
                    sum_ps = emit_sums(k - 1)
                    emit_norm(k - 1, sum_ps, pv_ps)
                # proj(j) spread as one st-quarter per slot (slots j*4+4 ..
                # j*4+7) so every mid-run slot carries the same PE load and
                # the PE never outpaces the exp stream (a >3.4us PE gap
                # re-throttles the HAM clock gate). The last group runs
                # monolithically in slot 16: dense back-to-back matmuls keep
                # the clock warm through the tail.
                if 4 <= k < 16:
                    emit_proj((k - 4) // 4, [(k - 4) % 4])
                if k == 16:
                    emit_proj(3, [0, 1, 2, 3], drain_engine="scalar")

    nc.compile()
    return nc


def _get_nc():
    global _COMPILED
    if _COMPILED is None:
        _COMPILED = _build()
    return _COMPILED


def _shard_inputs(q, k, v, Wc, bc):
    in_maps = []
    for c in range(8):
        b, g = divmod(c, 4)
        qT = np.ascontiguousarray(
            q[b][:, g * 512:(g + 1) * 512].reshape(S, HEADS_PER_GROUP, P).transpose(2, 1, 0)
        ).astype(ml_dtypes.bfloat16)
        kT = np.ascontiguousarray(k[b][:, g * P:(g + 1) * P].T).astype(ml_dtypes.bfloat16)
        # v prepacked to the SBUF layout [p, tile, hd]: (p, n, d) = v[n*128+p, d]
        vv = np.ascontiguousarray(
            v[b][:, g * P:(g + 1) * P].reshape(NT, P, P).transpose(1, 0, 2)
        ).astype(ml_dtypes.bfloat16)
        # wT prepacked to [p, chunk, out]: (p, n, o) = Wc[o, g*512 + n*128 + p]
        wT = np.ascontiguousarray(
            Wc[:, g * 512:(g + 1) * 512].T.reshape(HEADS_PER_GROUP, P, D_MODEL).transpose(1, 0, 2)
        ).astype(ml_dtypes.bfloat16)
        in_maps.append({"qT": qT, "kT": kT, "v": vv, "wT": wT})
    return in_maps


def _run(inputs, trace=False):
    q = np.asarray(inputs["q"], dtype=np.float32)
    k = np.asarray(inputs["k"], dtype=np.float32)
    v = np.asarray(inputs["v"], dtype=np.float32)
    Wc = np.asarray(inputs["Wc"], dtype=np.float32)
    bc = np.asarray(inputs["bc"], dtype=np.float32)

    nc = _get_nc()
    in_maps = _shard_inputs(q, k, v, Wc, bc)
    res = run_bass_kernel_spmd(nc, in_maps, list(range(8)), trace=trace)

    out = np.empty((B, S, D_MODEL), dtype=np.float32)
    for b in range(B):
        acc = res.results[4 * b]["out"].astype(np.float32).copy()
        for g in range(1, 4):
            acc += res.results[4 * b + g]["out"]
        out[b] = acc + bc[None, :]
    return out, res


def kernel(**inputs):
    out, _ = _run(inputs, trace=False)
    return out
